# revision 1
# baseline (speedup 1.0000x reference)
# Trainium2 Bass kernel for DeepSeek-style sparse attention.
# Self-contained: hardcodes shapes from the problem spec.
#   x [1, 2048, 768]; Wq/Wk/Wv/Wo [768, 768]; biases [768]; Ws [12, 768]; bs [12]
# Strategy: row-shard the 2048 query positions across 8 cores (256 rows each).
# Each core redundantly computes full K/V projections from a (per-core
# column-rotated) copy of x^T, so no collectives are needed. Three sparse
# attention branches (local band / learned top-k / global) are evaluated from
# one dense exp(S^T) per head:
#   - top-k: column mask folded into V (E @ (m*v)), mask from a tiny phase-A
#     token-score kernel + host argpartition between the two NEFF launches.
#   - local band: per-core rotation puts each core's 640-wide band in t-chunks
#     0..5; a host-built 0/1 mask is applied to E^T before a 6-chunk matmul.
#   - global (first 16 tokens): separate tiny k/v path from the unrotated
#     first 16 columns of x (uniform across cores despite the rotation).
# Matmuls run as float32r (TF32-like, 4x faster than fp32 for N>=256).
import sys
import numpy as np
import ml_dtypes

sys.path.insert(0, "/opt/trn_rl_repo")

import concourse.bass as bass
from concourse import bacc
import concourse.mybir as mybir
from concourse.tile import TileContext
from concourse.bass_utils import run_bass_kernel_spmd

S = 2048
D = 768
H = 12
DH = 64
NCORES = 8
RPC = S // NCORES          # 256 query rows per core
NCH = S // 128             # 16 t-chunks
ECH = D // 128             # 6 embedding chunks
TOPK = 256
NG = 16
LWH = 256                  # local window half-width
SCALE = 1.0 / np.sqrt(DH)
F32 = mybir.dt.float32
F32R = mybir.dt.float32r
BF16 = mybir.dt.bfloat16


def _patch_tile_drain():
    """This walrus build rejects sem-waits on Drain instructions ("Too many
    sync wait commands"). Emit the tail waits as individual SemWait ops on
    the sync engine instead, then a bare drain."""
    if getattr(TileContext, "_drain_patched", False):
        return

    def _drain_and_barrier(self, tick_clock, wait_clock):
        nc = self.nc
        clock = tick_clock.global_clock
        for proc, handle in sorted(self.sems.allocated().items()):
            tick = clock[proc]
            if tick <= 0:
                continue
            mult = 16 if "DMA" in handle.name else 1
            nc.sync.wait_ge(handle, tick * mult)
        nc.sync.drain()
        nc.all_engine_barrier()
        popped = nc._tile_sem_poison_stack.pop()
        assert popped is self._sem_poison
        nc.clear_and_free_semaphores(list(self.sems.allocated().values()))
        nc.all_engine_barrier()

    TileContext._drain_and_barrier = _drain_and_barrier
    TileContext._drain_patched = True


def _build_phase_a():
    """ts[h, t] = (Ws @ x^T + bs)[h, t] on one core, plain fp32."""
    nc = bacc.Bacc()
    xT = nc.declare_dram_parameter("xT", [D, S], F32, isOutput=False)
    WsT = nc.declare_dram_parameter("WsT", [D, H], F32, isOutput=False)
    bs_row = nc.declare_dram_parameter("bs_row", [1, H], F32, isOutput=False)
    ts = nc.declare_dram_parameter("ts", [H, S], F32, isOutput=True)
    xT_r = xT.rearrange("(c p) t -> c p t", p=128)
    WsT_r = WsT.rearrange("(c p) h -> c p h", p=128)

    with TileContext(nc) as tc:
        with (
            tc.tile_pool(name="sb", bufs=1) as sb,
            tc.tile_pool(name="ps", bufs=2, space="PSUM") as ps,
        ):
            xT_sb = sb.tile([128, ECH, S], F32)
            WsT_sb = sb.tile([128, ECH, H], F32)
            ones = sb.tile([1, 512], F32)
            bs_sb = sb.tile([1, H], F32)
            nc.vector.memset(ones, 1.0)
            nc.sync.dma_start(out=bs_sb, in_=bs_row[:, :])
            for ec in range(ECH):
                nc.sync.dma_start(out=xT_sb[:, ec, :], in_=xT_r[ec])
                nc.sync.dma_start(out=WsT_sb[:, ec, :], in_=WsT_r[ec])
            ts_sb = sb.tile([H, S], F32)
            for t4 in range(4):
                acc = ps.tile([H, 512], F32)
                for ec in range(ECH):
                    nc.tensor.matmul(
                        acc,
                        WsT_sb[:, ec, :],
                        xT_sb[:, ec, 512 * t4 : 512 * (t4 + 1)],
                        start=(ec == 0),
                        stop=False,
                    )
                nc.tensor.matmul(acc, bs_sb, ones, start=False, stop=True)
                nc.vector.tensor_copy(ts_sb[:, 512 * t4 : 512 * (t4 + 1)], acc)
            nc.sync.dma_start(out=ts[:, :], in_=ts_sb)
    nc.finalize()
    return nc


def _build_phase_b():
    """Per-core attention kernel. Query rows [c*256, (c+1)*256); t-columns of
    all per-t tensors are rotated left by 128*i0u(c) so the local band always
    occupies rotated t-chunks 0..5."""
    nc = bacc.Bacc()
    xTr = nc.declare_dram_parameter("xTr", [D, S], F32R, isOutput=False)
    xTq = nc.declare_dram_parameter("xTq", [D, RPC], F32R, isOutput=False)
    xTg = nc.declare_dram_parameter("xTg", [D, NG], F32R, isOutput=False)
    WqT = nc.declare_dram_parameter("WqT", [D, D], F32R, isOutput=False)
    WkT = nc.declare_dram_parameter("WkT", [D, D], F32R, isOutput=False)
    WvT = nc.declare_dram_parameter("WvT", [D, D], F32R, isOutput=False)
    WoT = nc.declare_dram_parameter("WoT", [H, DH, D], F32R, isOutput=False)
    bq_r = nc.declare_dram_parameter("bq_r", [1, D], F32R, isOutput=False)
    bk_r = nc.declare_dram_parameter("bk_r", [1, D], F32R, isOutput=False)
    bv_r = nc.declare_dram_parameter("bv_r", [1, D], F32R, isOutput=False)
    bo_r = nc.declare_dram_parameter("bo_r", [1, D], F32R, isOutput=False)
    M6 = nc.declare_dram_parameter("M6", [128, 6, RPC], BF16, isOutput=False)
    tkm = nc.declare_dram_parameter("tkm", [128, NCH, H], F32, isOutput=False)
    yT = nc.declare_dram_parameter("yT", [D, RPC], F32, isOutput=True)

    xTr_p = xTr.rearrange("(c p) t -> p c t", p=128)
    xTq_p = xTq.rearrange("(c p) t -> p c t", p=128)
    xTg_p = xTg.rearrange("(c p) t -> p c t", p=128)
    WqT_p = WqT.rearrange("(c p) d -> p c d", p=128)
    WkT_p = WkT.rearrange("(c p) d -> p c d", p=128)
    WvT_p = WvT.rearrange("(c p) d -> p c d", p=128)
    WoT_p = WoT.rearrange("h p d -> p h d")
    yT_p = yT.rearrange("(c p) t -> p c t", p=128)
    Exp = mybir.ActivationFunctionType.Exp

    with TileContext(nc) as tc, nc.allow_low_precision(reason="tf32/bf16 validated vs reference"):
        with tc.tile_pool(name="perm", bufs=1) as perm:
            kT_sb = perm.tile([128, ECH, S], F32R)
            kTg_sb = perm.tile([128, ECH, NG], F32R)
            v_sb = perm.tile([128, NCH, H, DH + 1], BF16)
            vg_sb = perm.tile([NG, H, DH + 1], BF16)
            qT_sb = perm.tile([128, ECH, RPC], F32R)
            tkm_sb = perm.tile([128, NCH, H], F32)
            M6_sb = perm.tile([128, 6, RPC], BF16)
            attnT_sb = perm.tile([DH, H, RPC], F32R)
            yT_sb = perm.tile([128, ECH, RPC], F32)
            ones = perm.tile([1, 512], F32R)
            ones65 = perm.tile([DH + 1, DH, ], F32R)
            bq_sb = perm.tile([1, D], F32R)
            bk_sb = perm.tile([1, D], F32R)
            bv_sb = perm.tile([1, D], F32R)
            bo_sb = perm.tile([1, D], F32R)
            onesf = perm.tile([DH + 1, 512], F32)
            nc.vector.memset(onesf, 1.0)
            nc.vector.tensor_copy(ones, onesf[0:1, :])
            nc.vector.tensor_copy(ones65, onesf[:, 0:DH])
            nc.vector.memset(v_sb, 1.0)
            nc.vector.memset(vg_sb, 1.0)
            nc.sync.dma_start(out=tkm_sb, in_=tkm[:, :, :])
            nc.sync.dma_start(out=M6_sb, in_=M6[:, :, :])
            for t, d in ((bq_sb, bq_r), (bk_sb, bk_r), (bv_sb, bv_r), (bo_sb, bo_r)):
                nc.sync.dma_start(out=t, in_=d[:, :])

            with (
                tc.tile_pool(name="xin", bufs=1) as xin,
                tc.tile_pool(name="pj_ps", bufs=4, space="PSUM") as pj_ps,
            ):
                xTr_sb = xin.tile([128, ECH, S], F32R)
                xTq_sb = xin.tile([128, ECH, RPC], F32R)
                xTg_sb = xin.tile([128, ECH, NG], F32R)
                nc.sync.dma_start(out=xTr_sb, in_=xTr_p)
                nc.sync.dma_start(out=xTq_sb, in_=xTq_p)
                nc.sync.dma_start(out=xTg_sb, in_=xTg_p)

                # ---- V projection (v natural [t, h, dh], +ones col) ----
                with tc.tile_pool(name="vw", bufs=1) as vw:
                    WvT_sb = vw.tile([128, ECH, D], F32R)
                    nc.sync.dma_start(out=WvT_sb, in_=WvT_p)
                    for tcn in range(NCH):
                        for half, (v0, v1) in enumerate(((0, 512), (512, 768))):
                            vn = v1 - v0
                            vp = pj_ps.tile([128, 512], F32, tag="pj")
                            for ec in range(ECH):
                                nc.tensor.matmul(
                                    vp[:, :vn],
                                    xTr_sb[:, ec, 128 * tcn : 128 * (tcn + 1)],
                                    WvT_sb[:, ec, v0:v1],
                                    start=(ec == 0), stop=False,
                                )
                            nc.tensor.matmul(
                                vp[:, :vn], ones[:, :128],
                                bv_sb[:, v0:v1], start=False, stop=True,
                            )
                            h0 = 0 if half == 0 else 8
                            nc.any.tensor_copy(
                                v_sb[:, tcn, h0 : h0 + vn // DH, 0:DH],
                                vp[:, :vn].rearrange("p (h d) -> p h d", d=DH),
                            )
                    # vg (first NG unrotated tokens)
                    for half, (v0, v1) in enumerate(((0, 512), (512, 768))):
                        vn = v1 - v0
                        vp = pj_ps.tile([128, 512], F32, tag="pj")
                        for ec in range(ECH):
                            nc.tensor.matmul(
                                vp[:NG, :vn], xTg_sb[:, ec, :],
                                WvT_sb[:, ec, v0:v1],
                                start=(ec == 0), stop=False,
                            )
                        nc.tensor.matmul(
                            vp[:NG, :vn], ones[:, :NG],
                            bv_sb[:, v0:v1], start=False, stop=True,
                        )
                        h0 = 0 if half == 0 else 8
                        nc.any.tensor_copy(
                            vg_sb[:, h0 : h0 + vn // DH, 0:DH],
                            vp[:NG, :vn].rearrange("p (h d) -> p h d", d=DH),
                        )

                # ---- K^T, K^T-global, Q^T projections ----
                with tc.tile_pool(name="kw", bufs=2) as kw:
                    for dc in range(ECH):
                        wk = kw.tile([128, ECH, 128], F32R, tag="wk")
                        wq = kw.tile([128, ECH, 128], F32R, tag="wq")
                        nc.sync.dma_start(out=wk, in_=WkT_p[:, :, 128 * dc : 128 * (dc + 1)])
                        nc.sync.dma_start(out=wq, in_=WqT_p[:, :, 128 * dc : 128 * (dc + 1)])
                        for t4 in range(4):
                            kp = pj_ps.tile([128, 512], F32, tag="pj")
                            for ec in range(ECH):
                                nc.tensor.matmul(
                                    kp, wk[:, ec, :],
                                    xTr_sb[:, ec, 512 * t4 : 512 * (t4 + 1)],
                                    start=(ec == 0), stop=False,
                                )
                            nc.tensor.matmul(
                                kp, bk_sb[:, 128 * dc : 128 * (dc + 1)],
                                ones[:, :512], start=False, stop=True,
                            )
                            nc.any.tensor_copy(kT_sb[:, dc, 512 * t4 : 512 * (t4 + 1)], kp)
                        kgp = pj_ps.tile([128, 512], F32, tag="pj")
                        for ec in range(ECH):
                            nc.tensor.matmul(
                                kgp[:, :NG], wk[:, ec, :], xTg_sb[:, ec, :],
                                start=(ec == 0), stop=False,
                            )
                        nc.tensor.matmul(
                            kgp[:, :NG], bk_sb[:, 128 * dc : 128 * (dc + 1)],
                            ones[:, :NG], start=False, stop=True,
                        )
                        nc.any.tensor_copy(kTg_sb[:, dc, :], kgp[:, :NG])
                        qp = pj_ps.tile([128, 512], F32, tag="pj")
                        for ec in range(ECH):
                            nc.tensor.matmul(
                                qp[:, :RPC], wq[:, ec, :],
                                xTq_sb[:, ec, :],
                                start=(ec == 0), stop=False,
                            )
                        nc.tensor.matmul(
                            qp[:, :RPC], bq_sb[:, 128 * dc : 128 * (dc + 1)],
                            ones[:, :RPC], start=False, stop=True,
                        )
                        nc.any.tensor_copy(qT_sb[:, dc, :], qp[:, :RPC])

            # ---- per-head attention ----
            with (
                tc.tile_pool(name="attn", bufs=2) as attn,
                tc.tile_pool(name="one_sb", bufs=1) as one_sb,
                tc.tile_pool(name="st_ps", bufs=2, space="PSUM") as st_ps,
                tc.tile_pool(name="av_ps", bufs=1, space="PSUM") as av_ps,
                tc.tile_pool(name="ms_ps", bufs=1, space="PSUM") as ms_ps,
            ):
                for h in range(H):
                    dc, hp = h // 2, (h % 2) * 64
                    kTh = kT_sb[hp : hp + 64, dc, :]
                    qTh = qT_sb[hp : hp + 64, dc, :]
                    ET = attn.tile([128, NCH, RPC], BF16, tag="ET")
                    for rnd in range(4):
                        stp = st_ps.tile([128, 4, RPC], F32, tag="st")
                        for j in range(4):
                            i = rnd * 4 + j
                            nc.tensor.matmul(
                                stp[:, j, :],
                                kTh[:, 128 * i : 128 * (i + 1)],
                                qTh, start=True, stop=True,
                            )
                        nc.scalar.activation(
                            ET[:, 4 * rnd : 4 * (rnd + 1), :], stp, Exp, scale=SCALE
                        )
                    stg = ms_ps.tile([64, 3, RPC], F32, tag="ms")
                    nc.tensor.matmul(
                        stg[:NG, 0, :], kTg_sb[hp : hp + 64, dc, :],
                        qTh, start=True, stop=True,
                    )
                    ETg = attn.tile([NG, RPC], BF16, tag="ETg")
                    nc.scalar.activation(ETg, stg[:NG, 0, :], Exp, scale=SCALE)
                    vm = attn.tile([128, NCH, DH + 1], BF16, tag="vm")
                    for i in range(NCH):
                        nc.vector.tensor_scalar_mul(
                            vm[:, i, :], v_sb[:, i, h, :], tkm_sb[:, i, h : h + 1]
                        )
                    EB = attn.tile([128, 6, RPC], BF16, tag="EB")
                    nc.vector.tensor_mul(EB, ET[:, 0:6, :], M6_sb)
                    av = av_ps.tile([128, 3, RPC], F32, tag="av")
                    for i in range(NCH):
                        nc.tensor.matmul(
                            av[0:65, 0, :], vm[:, i, :], ET[:, i, :],
                            start=(i == 0), stop=(i == NCH - 1),
                        )
                    for k in range(6):
                        nc.tensor.matmul(
                            av[0:65, 1, :], v_sb[:, k, h, :], EB[:, k, :],
                            start=(k == 0), stop=(k == 5),
                        )
                    nc.tensor.matmul(
                        av[0:65, 2, :], vg_sb[:, h, :], ETg, start=True, stop=True,
                    )
                    sums = attn.tile([DH + 1, 3, RPC], F32, tag="sums")
                    nc.vector.tensor_scalar_mul(
                        sums[DH : DH + 1, :, :], av[DH : DH + 1, :, :], 3.0
                    )
                    rin = attn.tile([DH + 1, 3, RPC], F32R, tag="rin")
                    nc.vector.reciprocal(
                        rin[DH : DH + 1, :, :], sums[DH : DH + 1, :, :]
                    )
                    rbc = ms_ps.tile([64, 3, RPC], F32, tag="ms")
                    for b in range(3):
                        nc.tensor.matmul(
                            rbc[:, b, :], ones65[DH : DH + 1, :],
                            rin[DH : DH + 1, b, :], start=True, stop=True,
                        )
                    rbs = attn.tile([64, 3, RPC], F32, tag="rbs")
                    nc.vector.tensor_copy(rbs, rbc)
                    acc = attnT_sb[:, h, :]
                    tmp = attn.tile([64, RPC], F32, tag="tmp")
                    nc.vector.tensor_mul(acc, av[0:64, 0, :], rbs[:, 0, :])
                    nc.vector.tensor_mul(tmp, av[0:64, 1, :], rbs[:, 1, :])
                    nc.vector.tensor_add(acc, acc, tmp)
                    nc.vector.tensor_mul(tmp, av[0:64, 2, :], rbs[:, 2, :])
                    nc.vector.tensor_add(acc, acc, tmp)

            # ---- output projection yT = WoT.T @ attnT + bo ----
            with (
                tc.tile_pool(name="wo", bufs=2) as wo_pool,
                tc.tile_pool(name="yt_ps", bufs=2, space="PSUM") as yt_ps,
            ):
                for dc in range(ECH):
                    wo = wo_pool.tile([DH, H, 128], F32R, tag="wo")
                    nc.sync.dma_start(out=wo, in_=WoT_p[:, :, 128 * dc : 128 * (dc + 1)])
                    yp = yt_ps.tile([128, RPC], F32, tag="yt")
                    for h in range(H):
                        nc.tensor.matmul(
                            yp, wo[:, h, :],
                            attnT_sb[:, h, :],
                            start=(h == 0), stop=False,
                        )
                    nc.tensor.matmul(
                        yp, bo_sb[:, 128 * dc : 128 * (dc + 1)],
                        ones[:, :RPC], start=False, stop=True,
                    )
                    nc.any.tensor_copy(yT_sb[:, dc, :], yp)
                    nc.sync.dma_start(out=yT_p[:, dc, :], in_=yT_sb[:, dc, :])
    nc.finalize()
    return nc


_PROGS = {}
TRACE = False
LAST_EXEC_NS = {}


def _get_progs():
    if "a" not in _PROGS:
        _PROGS["a"] = _build_phase_a()
        _PROGS["b"] = _build_phase_b()
    return _PROGS["a"], _PROGS["b"]


def _band_mask(c):
    i0u = min(max(2 * c - 2, 0), 10)
    r0 = c * RPC
    p = np.arange(128)[:, None, None]
    k = np.arange(6)[None, :, None]
    sl = np.arange(RPC)[None, None, :]
    t = 128 * (i0u + k) + p
    sg = r0 + sl
    return (np.abs(t - sg) <= LWH).astype(np.float32), i0u


def kernel(**inputs):
    x = np.ascontiguousarray(inputs["x"][0], np.float32)        # [S, D]
    xT = np.ascontiguousarray(x.T)                              # [D, S]
    nc_a, nc_b = _get_progs()

    # phase A: token scores on core 0
    in_a = {
        "xT": xT,
        "WsT": np.ascontiguousarray(inputs["Ws"].T, np.float32),
        "bs_row": np.ascontiguousarray(inputs["bs"][None, :], np.float32),
    }
    ra = run_bass_kernel_spmd(nc_a, [in_a], [0], trace=TRACE)
    ts = ra.results[0]["ts"]  # [H, S]
    LAST_EXEC_NS["phase_a"] = ra.exec_time_ns

    # host: top-k column mask per head
    tkm = np.zeros((H, S), np.float32)
    for h in range(H):
        tkm[h, np.argpartition(-ts[h], TOPK)[:TOPK]] = 1.0

    shared = {
        "WqT": np.ascontiguousarray(inputs["Wq"].T, np.float32),
        "WkT": np.ascontiguousarray(inputs["Wk"].T, np.float32),
        "WvT": np.ascontiguousarray(inputs["Wv"].T, np.float32),
        "WoT": np.ascontiguousarray(inputs["Wo"].T, np.float32).reshape(H, DH, D),
        "bq_r": np.ascontiguousarray(inputs["bq"][None, :], np.float32),
        "bk_r": np.ascontiguousarray(inputs["bk"][None, :], np.float32),
        "bv_r": np.ascontiguousarray(inputs["bv"][None, :], np.float32),
        "bo_r": np.ascontiguousarray(inputs["bo"][None, :], np.float32),
        "xTg": np.ascontiguousarray(xT[:, :NG]),
    }
    in_maps = []
    for c in range(NCORES):
        M6, i0u = _band_mask(c)
        rot = np.roll(xT, -128 * i0u, axis=1)
        tkm_rot = np.roll(tkm, -128 * i0u, axis=1)
        tkm_r = np.ascontiguousarray(
            tkm_rot.reshape(H, NCH, 128).transpose(2, 1, 0), np.float32
        )
        in_maps.append(dict(
            shared,
            xTr=np.ascontiguousarray(rot),
            xTq=np.ascontiguousarray(xT[:, c * RPC : (c + 1) * RPC]),
            M6=np.ascontiguousarray(M6.astype(ml_dtypes.bfloat16)),
            tkm=tkm_r,
        ))
    res = run_bass_kernel_spmd(nc_b, in_maps, list(range(NCORES)), trace=TRACE)
    LAST_EXEC_NS["phase_b"] = res.exec_time_ns
    out = np.empty((S, D), np.float32)
    for c in range(NCORES):
        out[c * RPC : (c + 1) * RPC] = res.results[c]["yT"].T
    return out.reshape(1, S, D)



# revision 25
# speedup vs baseline: 1.2571x; 1.2571x over previous
# Trainium2 Bass kernel for DeepSeek-style sparse attention.
# Self-contained: hardcodes shapes from the problem spec.
#   x [1, 2048, 768]; Wq/Wk/Wv/Wo [768, 768]; biases [768]; Ws [12, 768]; bs [12]
# Strategy: row-shard the 2048 query positions across 8 cores (256 rows each).
# Each core redundantly computes full K/V projections from a (per-core
# column-rotated) copy of x^T, so no collectives are needed. Three sparse
# attention branches (local band / learned top-k / global) are evaluated from
# one dense exp(S^T) per head:
#   - top-k: column mask folded into V (E @ (m*v)), mask from a tiny phase-A
#     token-score kernel + host argpartition between the two NEFF launches.
#   - local band: per-core rotation puts each core's 640-wide band in t-chunks
#     0..5; a host-built 0/1 mask is applied to E^T before a 6-chunk matmul.
#   - global (first 16 tokens): separate tiny k/v path from the unrotated
#     first 16 columns of x (uniform across cores despite the rotation).
# Matmuls run as float32r (TF32-like, 4x faster than fp32 for N>=256).
import sys
import numpy as np
import ml_dtypes

sys.path.insert(0, "/opt/trn_rl_repo")

import concourse.bass as bass
from concourse import bacc
import concourse.mybir as mybir
from concourse.tile import TileContext
from concourse.bass_utils import run_bass_kernel_spmd

S = 2048
D = 768
H = 12
DH = 64
NCORES = 8
RPC = S // NCORES          # 256 query rows per core
NCH = S // 128             # 16 t-chunks
ECH = D // 128             # 6 embedding chunks
TOPK = 256
NG = 16
LWH = 256                  # local window half-width
SCALE = 1.0 / np.sqrt(DH)
F32 = mybir.dt.float32
F32R = mybir.dt.float32r
BF16 = mybir.dt.bfloat16


def _patch_tile_drain():
    """This walrus build rejects sem-waits on Drain instructions ("Too many
    sync wait commands"). Emit the tail waits as individual SemWait ops on
    the sync engine instead, then a bare drain."""
    if getattr(TileContext, "_drain_patched", False):
        return

    def _drain_and_barrier(self, tick_clock, wait_clock):
        nc = self.nc
        clock = tick_clock.global_clock
        for proc, handle in sorted(self.sems.allocated().items()):
            tick = clock[proc]
            if tick <= 0:
                continue
            mult = 16 if "DMA" in handle.name else 1
            nc.sync.wait_ge(handle, tick * mult)
        nc.sync.drain()
        nc.all_engine_barrier()
        popped = nc._tile_sem_poison_stack.pop()
        assert popped is self._sem_poison
        nc.clear_and_free_semaphores(list(self.sems.allocated().values()))
        nc.all_engine_barrier()

    TileContext._drain_and_barrier = _drain_and_barrier
    TileContext._drain_patched = True


def _build_phase_a():
    """ts[h, t] = (Ws @ x^T + bs)[h, t] on one core, plain fp32."""
    nc = bacc.Bacc()
    xT = nc.declare_dram_parameter("xT", [D, S], BF16, isOutput=False)
    WsT = nc.declare_dram_parameter("WsT", [D, H], BF16, isOutput=False)
    bs_row = nc.declare_dram_parameter("bs_row", [1, H], F32, isOutput=False)
    ts = nc.declare_dram_parameter("ts", [H, S], F32, isOutput=True)
    xT_r = xT.rearrange("(c p) t -> c p t", p=128)
    WsT_r = WsT.rearrange("(c p) h -> c p h", p=128)

    with TileContext(nc) as tc, nc.allow_low_precision(reason="bf16 validated vs reference"):
        with (
            tc.tile_pool(name="sb", bufs=1) as sb,
            tc.tile_pool(name="ps", bufs=2, space="PSUM") as ps,
        ):
            xT_sb = sb.tile([128, ECH, S], BF16)
            WsT_sb = sb.tile([128, ECH, H], BF16)
            bs_sb = sb.tile([1, H], F32)
            # bias bs is a per-head constant shift: it cannot change each
            # head's top-k column selection, so it is loaded but not applied.
            nc.sync.dma_start(out=bs_sb, in_=bs_row[:, :])
            for ec in range(ECH):
                nc.sync.dma_start(out=xT_sb[:, ec, :], in_=xT_r[ec])
                nc.sync.dma_start(out=WsT_sb[:, ec, :], in_=WsT_r[ec])
            ts_sb = sb.tile([H, S], F32)
            for t4 in range(4):
                acc = ps.tile([H, 512], F32)
                for ec in range(ECH):
                    nc.tensor.matmul(
                        acc,
                        WsT_sb[:, ec, :],
                        xT_sb[:, ec, 512 * t4 : 512 * (t4 + 1)],
                        start=(ec == 0),
                        stop=(ec == ECH - 1),
                    )
                nc.vector.tensor_copy(ts_sb[:, 512 * t4 : 512 * (t4 + 1)], acc)
            nc.sync.dma_start(out=ts[:, :], in_=ts_sb)
    nc.finalize()
    return nc


def _build_phase_b():
    """Per-core attention kernel. Query rows [c*256, (c+1)*256); t-columns of
    all per-t tensors are rotated left by 128*i0u(c) so the local band always
    occupies rotated t-chunks 0..5."""
    nc = bacc.Bacc()
    xTr = nc.declare_dram_parameter("xTr", [D, S], BF16, isOutput=False)
    xTq = nc.declare_dram_parameter("xTq", [D, RPC], BF16, isOutput=False)
    xTg = nc.declare_dram_parameter("xTg", [D, NG], BF16, isOutput=False)
    WqT = nc.declare_dram_parameter("WqT", [D, D], BF16, isOutput=False)
    WkT = nc.declare_dram_parameter("WkT", [D, D], BF16, isOutput=False)
    WvT = nc.declare_dram_parameter("WvT", [D, D], BF16, isOutput=False)
    WoT = nc.declare_dram_parameter("WoT", [H, DH, D], F32R, isOutput=False)
    bv_r = nc.declare_dram_parameter("bv_r", [1, D], F32R, isOutput=False)
    bkT = nc.declare_dram_parameter("bkT", [128, ECH], F32, isOutput=False)
    bqT = nc.declare_dram_parameter("bqT", [128, ECH], F32, isOutput=False)
    boT = nc.declare_dram_parameter("boT", [128, ECH], F32, isOutput=False)
    M6 = nc.declare_dram_parameter("M6", [128, 6, RPC], BF16, isOutput=False)
    tkm = nc.declare_dram_parameter("tkm", [128, NCH, H], F32, isOutput=False)
    yT = nc.declare_dram_parameter("yT", [D, RPC], F32, isOutput=True)

    xTr_p = xTr.rearrange("(c p) t -> p c t", p=128)
    xTq_p = xTq.rearrange("(c p) t -> p c t", p=128)
    xTg_p = xTg.rearrange("(c p) t -> p c t", p=128)
    WqT_p = WqT.rearrange("(c p) d -> p c d", p=128)
    WkT_p = WkT.rearrange("(c p) d -> p c d", p=128)
    WvT_p = WvT.rearrange("(c p) d -> p c d", p=128)
    WoT_p = WoT.rearrange("h p d -> p h d")
    yT_p = yT.rearrange("(c p) t -> p c t", p=128)
    Exp = mybir.ActivationFunctionType.Exp
    Ident = mybir.ActivationFunctionType.Identity

    with TileContext(nc) as tc, nc.allow_low_precision(reason="tf32/bf16 validated vs reference"):
        with tc.tile_pool(name="perm", bufs=1) as perm:
            kT_sb = perm.tile([128, ECH, S], BF16)
            kTg_sb = perm.tile([128, ECH, NG], BF16)
            v_sb = perm.tile([128, NCH, H, DH + 1], BF16)
            vg_sb = perm.tile([NG, H, DH + 1], BF16)
            qT_sb = perm.tile([128, ECH, RPC], BF16)
            tkm_sb = perm.tile([128, NCH, H], F32)
            M6_sb = perm.tile([128, 6, RPC], BF16)
            attnT_sb = perm.tile([DH, H, RPC], F32R)
            yT_sb = perm.tile([128, ECH, RPC], F32)
            ones = perm.tile([1, 512], F32R)
            ones65 = perm.tile([DH + 1, DH], F32R)
            bv_sb = perm.tile([1, D], F32R)
            bkT_sb = perm.tile([128, ECH], F32)
            bqT_sb = perm.tile([128, ECH], F32)
            boT_sb = perm.tile([128, ECH], F32)
            onesf = perm.tile([DH + 1, 512], F32)
            nc.vector.memset(onesf, 1.0)
            nc.vector.tensor_copy(ones, onesf[0:1, :])
            nc.vector.tensor_copy(ones65, onesf[:, 0:DH])
            nc.vector.memset(v_sb, 1.0)
            nc.vector.memset(vg_sb, 1.0)
            nc.sync.dma_start(out=tkm_sb, in_=tkm[:, :, :])
            nc.sync.dma_start(out=M6_sb, in_=M6[:, :, :])
            for t, d in ((bv_sb, bv_r), (bkT_sb, bkT), (bqT_sb, bqT), (boT_sb, boT)):
                nc.sync.dma_start(out=t, in_=d[:, :])

            with (
                tc.tile_pool(name="xin", bufs=1) as xin,
                tc.tile_pool(name="pj_ps", bufs=4, space="PSUM") as pj_ps,
            ):
                xTr_sb = xin.tile([128, ECH, S], BF16)
                xTq_sb = xin.tile([128, ECH, RPC], BF16)
                xTg_sb = xin.tile([128, ECH, NG], BF16)
                nc.sync.dma_start(out=xTr_sb, in_=xTr_p)
                nc.sync.dma_start(out=xTq_sb, in_=xTq_p)
                nc.sync.dma_start(out=xTg_sb, in_=xTg_p)

                # ---- V projection (v natural [t, h, dh], +ones col) ----
                with tc.tile_pool(name="vw", bufs=1) as vw:
                    WvT_sb = vw.tile([128, ECH, D], BF16)
                    nc.sync.dma_start(out=WvT_sb, in_=WvT_p)
                    for tcn in range(NCH):
                        for half, (v0, v1) in enumerate(((0, 512), (512, 768))):
                            vn = v1 - v0
                            vp = pj_ps.tile([128, 512], F32, tag="pj")
                            for ec in range(ECH):
                                nc.tensor.matmul(
                                    vp[:, :vn],
                                    xTr_sb[:, ec, 128 * tcn : 128 * (tcn + 1)],
                                    WvT_sb[:, ec, v0:v1],
                                    start=(ec == 0), stop=False,
                                )
                            nc.tensor.matmul(
                                vp[:, :vn], ones[:, :128],
                                bv_sb[:, v0:v1], start=False, stop=True,
                            )
                            h0 = 0 if half == 0 else 8
                            nc.any.tensor_copy(
                                v_sb[:, tcn, h0 : h0 + vn // DH, 0:DH],
                                vp[:, :vn].rearrange("p (h d) -> p h d", d=DH),
                            )
                    # vg (first NG unrotated tokens)
                    for half, (v0, v1) in enumerate(((0, 512), (512, 768))):
                        vn = v1 - v0
                        vp = pj_ps.tile([128, 512], F32, tag="pj")
                        for ec in range(ECH):
                            nc.tensor.matmul(
                                vp[:NG, :vn], xTg_sb[:, ec, :],
                                WvT_sb[:, ec, v0:v1],
                                start=(ec == 0), stop=False,
                            )
                        nc.tensor.matmul(
                            vp[:NG, :vn], ones[:, :NG],
                            bv_sb[:, v0:v1], start=False, stop=True,
                        )
                        h0 = 0 if half == 0 else 8
                        nc.any.tensor_copy(
                            vg_sb[:, h0 : h0 + vn // DH, 0:DH],
                            vp[:NG, :vn].rearrange("p (h d) -> p h d", d=DH),
                        )

                # ---- K^T, K^T-global, Q^T projections ----
                with tc.tile_pool(name="kw", bufs=2) as kw:
                    for dc in range(ECH):
                        wk = kw.tile([128, ECH, 128], BF16, tag="wk")
                        wq = kw.tile([128, ECH, 128], BF16, tag="wq")
                        nc.sync.dma_start(out=wk, in_=WkT_p[:, :, 128 * dc : 128 * (dc + 1)])
                        nc.sync.dma_start(out=wq, in_=WqT_p[:, :, 128 * dc : 128 * (dc + 1)])
                        for t4 in range(4):
                            kp = pj_ps.tile([128, 512], F32, tag="pj")
                            for ec in range(ECH):
                                nc.tensor.matmul(
                                    kp, wk[:, ec, :],
                                    xTr_sb[:, ec, 512 * t4 : 512 * (t4 + 1)],
                                    start=(ec == 0), stop=(ec == ECH - 1),
                                )
                            nc.scalar.activation(
                                kT_sb[:, dc, 512 * t4 : 512 * (t4 + 1)], kp,
                                Ident, bias=bkT_sb[:, dc : dc + 1],
                            )
                        kgp = pj_ps.tile([128, 512], F32, tag="pj")
                        for ec in range(ECH):
                            nc.tensor.matmul(
                                kgp[:, :NG], wk[:, ec, :], xTg_sb[:, ec, :],
                                start=(ec == 0), stop=(ec == ECH - 1),
                            )
                        nc.scalar.activation(
                            kTg_sb[:, dc, :], kgp[:, :NG],
                            Ident, bias=bkT_sb[:, dc : dc + 1],
                        )
                        qp = pj_ps.tile([128, 512], F32, tag="pj")
                        for ec in range(ECH):
                            nc.tensor.matmul(
                                qp[:, :RPC], wq[:, ec, :],
                                xTq_sb[:, ec, :],
                                start=(ec == 0), stop=(ec == ECH - 1),
                            )
                        nc.scalar.activation(
                            qT_sb[:, dc, :], qp[:, :RPC],
                            Ident, bias=bqT_sb[:, dc : dc + 1],
                        )

            # ---- per-head attention ----
            with (
                tc.tile_pool(name="attn", bufs=2) as attn,
                tc.tile_pool(name="one_sb", bufs=1) as one_sb,
                tc.tile_pool(name="st_ps", bufs=2, space="PSUM") as st_ps,
                tc.tile_pool(name="av_ps", bufs=2, space="PSUM") as av_ps,
                tc.tile_pool(name="ms_ps", bufs=1, space="PSUM") as ms_ps,
            ):
                for h in range(H):
                    dc, hp = h // 2, (h % 2) * 64
                    kTh = kT_sb[hp : hp + 64, dc, :]
                    qTh = qT_sb[hp : hp + 64, dc, :]
                    ET = attn.tile([128, NCH, RPC], BF16, tag="ET")
                    for rnd in range(8):
                        stp = st_ps.tile([128, 2, RPC], F32, tag="st")
                        for j in range(2):
                            i = rnd * 2 + j
                            nc.tensor.matmul(
                                stp[:, j, :],
                                kTh[:, 128 * i : 128 * (i + 1)],
                                qTh, start=True, stop=True,
                            )
                        nc.scalar.activation(
                            ET[:, 2 * rnd : 2 * (rnd + 1), :], stp, Exp, scale=SCALE
                        )
                    stg = ms_ps.tile([64, 3, RPC], F32, tag="ms")
                    nc.tensor.matmul(
                        stg[:NG, 0, :], kTg_sb[hp : hp + 64, dc, :],
                        qTh, start=True, stop=True,
                    )
                    ETg = attn.tile([NG, RPC], BF16, tag="ETg")
                    nc.scalar.activation(ETg, stg[:NG, 0, :], Exp, scale=SCALE)
                    vm = attn.tile([128, NCH, DH + 1], BF16, tag="vm")
                    for i in range(NCH):
                        nc.vector.tensor_scalar_mul(
                            vm[:, i, :], v_sb[:, i, h, :], tkm_sb[:, i, h : h + 1]
                        )
                    EB = attn.tile([128, 6, RPC], BF16, tag="EB")
                    nc.vector.tensor_mul(EB, ET[:, 0:6, :], M6_sb)
                    av = av_ps.tile([128, 3, RPC], F32, tag="av")
                    for i in range(NCH):
                        nc.tensor.matmul(
                            av[0:65, 0, :], vm[:, i, :], ET[:, i, :],
                            start=(i == 0), stop=(i == NCH - 1),
                        )
                    for k in range(6):
                        nc.tensor.matmul(
                            av[0:65, 1, :], v_sb[:, k, h, :], EB[:, k, :],
                            start=(k == 0), stop=(k == 5),
                        )
                    nc.tensor.matmul(
                        av[0:65, 2, :], vg_sb[:, h, :], ETg, start=True, stop=True,
                    )
                    sums = attn.tile([DH + 1, 3, RPC], F32, tag="sums")
                    nc.vector.tensor_scalar_mul(
                        sums[DH : DH + 1, :, :], av[DH : DH + 1, :, :], 3.0
                    )
                    rin = attn.tile([DH + 1, 3, RPC], F32R, tag="rin")
                    nc.vector.reciprocal(
                        rin[DH : DH + 1, :, :], sums[DH : DH + 1, :, :]
                    )
                    rbc = ms_ps.tile([64, 3, RPC], F32, tag="ms")
                    for b in range(3):
                        nc.tensor.matmul(
                            rbc[:, b, :], ones65[DH : DH + 1, :],
                            rin[DH : DH + 1, b, :], start=True, stop=True,
                        )
                    rbs = attn.tile([64, 3, RPC], F32, tag="rbs")
                    nc.any.tensor_copy(rbs, rbc)
                    prod = attn.tile([64, 3, RPC], F32, tag="prod")
                    nc.vector.tensor_mul(prod, av[0:64, :, :], rbs)
                    acc = attnT_sb[:, h, :]
                    nc.vector.tensor_add(acc, prod[:, 0, :], prod[:, 1, :])
                    nc.vector.tensor_add(acc, acc, prod[:, 2, :])

            # ---- output projection yT = WoT.T @ attnT + bo ----
            with (
                tc.tile_pool(name="wo", bufs=2) as wo_pool,
                tc.tile_pool(name="yt_ps", bufs=2, space="PSUM") as yt_ps,
            ):
                for dc in range(ECH):
                    wo = wo_pool.tile([DH, H, 128], F32R, tag="wo")
                    nc.sync.dma_start(out=wo, in_=WoT_p[:, :, 128 * dc : 128 * (dc + 1)])
                    yp = yt_ps.tile([128, RPC], F32, tag="yt")
                    for h in range(H):
                        nc.tensor.matmul(
                            yp, wo[:, h, :],
                            attnT_sb[:, h, :],
                            start=(h == 0), stop=(h == H - 1),
                        )
                    nc.scalar.activation(
                        yT_sb[:, dc, :], yp,
                        Ident, bias=boT_sb[:, dc : dc + 1],
                    )
                    nc.sync.dma_start(out=yT_p[:, dc, :], in_=yT_sb[:, dc, :])
    nc.finalize()
    return nc


_PROGS = {}
TRACE = False
LAST_EXEC_NS = {}


def _get_progs():
    if "a" not in _PROGS:
        _PROGS["a"] = _build_phase_a()
        _PROGS["b"] = _build_phase_b()
    return _PROGS["a"], _PROGS["b"]


def _band_mask(c):
    i0u = min(max(2 * c - 2, 0), 10)
    r0 = c * RPC
    p = np.arange(128)[:, None, None]
    k = np.arange(6)[None, :, None]
    sl = np.arange(RPC)[None, None, :]
    t = 128 * (i0u + k) + p
    sg = r0 + sl
    return (np.abs(t - sg) <= LWH).astype(np.float32), i0u


def kernel(**inputs):
    x = np.ascontiguousarray(inputs["x"][0], np.float32)        # [S, D]
    xT = np.ascontiguousarray(x.T)                              # [D, S]
    nc_a, nc_b = _get_progs()

    # phase A: token scores on core 0
    in_a = {
        "xT": np.ascontiguousarray(xT.astype(ml_dtypes.bfloat16)),
        "WsT": np.ascontiguousarray(inputs["Ws"].T.astype(ml_dtypes.bfloat16)),
        "bs_row": np.ascontiguousarray(inputs["bs"][None, :], np.float32),
    }
    ra = run_bass_kernel_spmd(nc_a, [in_a], [0], trace=TRACE)
    ts = ra.results[0]["ts"]  # [H, S]
    LAST_EXEC_NS["phase_a"] = ra.exec_time_ns

    # host: top-k column mask per head
    tkm = np.zeros((H, S), np.float32)
    for h in range(H):
        tkm[h, np.argpartition(-ts[h], TOPK)[:TOPK]] = 1.0

    bf = ml_dtypes.bfloat16
    xTb = xT.astype(bf)
    shared = {
        "WqT": np.ascontiguousarray(inputs["Wq"].T.astype(bf)),
        "WkT": np.ascontiguousarray(inputs["Wk"].T.astype(bf)),
        "WvT": np.ascontiguousarray(inputs["Wv"].T.astype(bf)),
        "WoT": np.ascontiguousarray(inputs["Wo"].T, np.float32).reshape(H, DH, D),
        "bv_r": np.ascontiguousarray(inputs["bv"][None, :], np.float32),
        "bkT": np.ascontiguousarray(inputs["bk"].reshape(ECH, 128).T, np.float32),
        "bqT": np.ascontiguousarray(inputs["bq"].reshape(ECH, 128).T, np.float32),
        "boT": np.ascontiguousarray(inputs["bo"].reshape(ECH, 128).T, np.float32),
        "xTg": np.ascontiguousarray(xTb[:, :NG]),
    }
    in_maps = []
    for c in range(NCORES):
        M6, i0u = _band_mask(c)
        rot = np.roll(xTb, -128 * i0u, axis=1)
        tkm_rot = np.roll(tkm, -128 * i0u, axis=1)
        tkm_r = np.ascontiguousarray(
            tkm_rot.reshape(H, NCH, 128).transpose(2, 1, 0), np.float32
        )
        in_maps.append(dict(
            shared,
            xTr=np.ascontiguousarray(rot),
            xTq=np.ascontiguousarray(xTb[:, c * RPC : (c + 1) * RPC]),
            M6=np.ascontiguousarray(M6.astype(bf)),
            tkm=tkm_r,
        ))
    res = run_bass_kernel_spmd(nc_b, in_maps, list(range(NCORES)), trace=TRACE)
    LAST_EXEC_NS["phase_b"] = res.exec_time_ns
    out = np.empty((S, D), np.float32)
    for c in range(NCORES):
        out[c * RPC : (c + 1) * RPC] = res.results[c]["yT"].T
    return out.reshape(1, S, D)

